# revision 86
# baseline (speedup 1.0000x reference)
"""Multi-head attention (B=2, S=2048, D=1024, H=16) on 8 TRN2 NeuronCores.

Sharding: tensor-parallel over heads (2 heads per core) for QKV projections and
attention; AllToAll repartitions the context to token-parallel for the output
projection (avoids the expensive AllReduce).

Key compaction: masked key positions contribute exactly 0 to softmax numerator
and denominator (exp(-1e9) == 0 in fp32), so the host gathers only unmasked
key/value tokens (padded to a 128 multiple; pad columns get -1e9 bias), which
cuts K/V projection, logits, exp and PV work proportionally.

Precision: activations and weights are converted to bf16 on the host (matmuls
accumulate in fp32 PSUM), halving the dominant HBM input traffic.

Schedule notes: x tiles stream in one large DMA per 512-token chunk issued
from the gpsimd queue (the sync queue head-blocks on in-flight DMAs); logits
for both heads land in one two-bank PSUM tile so a single Exp activation
covers them; PV matmuls lag logits by one key block so the tensor engine is
not stalled on the scalar engine's exp; projection chunks are spread across
all eight attention blocks as tensor-engine filler; the final attention block
is processed in four 128-query quarters, each feeding its own small AllToAll,
so the collectives and the ctx re-gather pipeline behind the remaining
quarters and the output projection starts with a warm PE array.

kernel(**inputs) takes the FULL inputs (as produced by setup_inputs()) and
returns the FULL [2, 2048, 1024] output.
"""
import math

import numpy as np

import concourse.mybir as mybir
import concourse.tile as tile
from concourse import bacc, bass_utils

# problem constants (hardcoded per contract)
B, S, D, H = 2, 2048, 1024, 16
T = B * S                 # 4096 flattened query tokens
DEPTH = D // H            # 64
N_CORES = 8
HL = 2 * DEPTH            # 128 local head dims per core (2 heads)
TCHUNK = T // N_CORES     # 512 tokens per core for the output projection
NDT = D // 128            # 8 contraction tiles of 128
NQB = S // 512            # 4 query blocks of 512 per batch

F32 = mybir.dt.float32
BF16 = mybir.dt.bfloat16
FP8 = mybir.dt.float8e4
NPBF16 = mybir.dt.np(BF16)
NPFP8 = mybir.dt.np(FP8)


def build_nc(kblocks: int = 16, collective: bool = True, num_devices: int = N_CORES,
             dump: bool = False, clean_upto=(0, 0),
             phases=("kproj", "vproj", "qproj", "attn", "a2a", "out"), reps: int = 1):
    """kblocks: compacted key blocks (of 128) per batch; 16 = uncompacted.
    clean_upto[b]: key blocks below this index have no masked positions in
    batch b, so their exp bias is exactly zero and one activation can cover a
    whole group of blocks."""
    phases = set(phases)
    SK = kblocks * 128        # compacted keys per batch
    TK = B * SK               # total compacted key tokens
    NCON = 2 + HL + B * kblocks   # packed f32 consts: bq | bk | bvb | maskb
    nc = bacc.Bacc(
        "TRN2", target_bir_lowering=False, debug=False, num_devices=num_devices
    )

    # ---- I/O ----
    xqT = nc.dram_tensor("xqT", [D, T], BF16, kind="ExternalInput")   # query^T
    xkT = nc.dram_tensor("xkT", [D, TK], BF16, kind="ExternalInput")  # compacted key^T
    xvT = nc.dram_tensor("xvT", [D, TK], BF16, kind="ExternalInput")  # compacted value^T
    # packed per-core weight slices [wk | wv | wq], host-prearranged to the
    # SBUF layout [128, NDT, 3*HL] so the load is one contiguous DMA
    wkvq = nc.dram_tensor("wkvq", [128, NDT * 3 * HL], BF16, kind="ExternalInput")
    wo = nc.dram_tensor("wo", [128, NDT * D], BF16, kind="ExternalInput")
    consts = nc.dram_tensor("consts", [128, NCON], F32, kind="ExternalInput")
    bob = nc.dram_tensor("bob", [128, D], F32, kind="ExternalInput")  # bo bcast
    out = nc.dram_tensor("out", [TCHUNK, D], F32, kind="ExternalOutput")

    if dump:
        d_qT = nc.dram_tensor("d_qT", [128, T], F32, kind="ExternalOutput")
        d_kT = nc.dram_tensor("d_kT", [128, TK], F32, kind="ExternalOutput")
        d_v = nc.dram_tensor("d_v", [128, TK // 128, 130], F32, kind="ExternalOutput")
        d_ctxT = nc.dram_tensor("d_ctxT", [128, T], F32, kind="ExternalOutput")

    NA2A = 4           # pipelined A2A rounds, one per 128-token quarter
    TH = TCHUNK // NA2A
    a2a_in = [nc.dram_tensor(f"a2a_in{h}", [N_CORES, HL, TH], BF16)
              for h in range(NA2A)]
    a2a_out = [nc.dram_tensor(f"a2a_out{h}", [N_CORES, HL, TH], BF16)
               for h in range(NA2A)]

    with tile.TileContext(nc) as tc:
        with (
            tc.tile_pool(name="w", bufs=1) as wp,        # weights / constants
            tc.tile_pool(name="big", bufs=1) as bigp,    # persistent activations
            tc.tile_pool(name="io", bufs=6) as iop,      # streaming tiles
            tc.tile_pool(name="ps", bufs=1, space="PSUM") as ps,
        ):
            AFT = mybir.ActivationFunctionType
            # ---- constants needed early (packed loads) ----
            WSTRIDE = NDT * HL
            wkvq_sb = wp.tile([128, 3, NDT, HL], BF16)

            def load_w(wi, slot, eng, nloads=1):
                hs = NDT // nloads
                for l in range(nloads):
                    eng.dma_start(
                        wkvq_sb[:, slot, l * hs:(l + 1) * hs],
                        wkvq.ap()[:, wi * WSTRIDE + l * hs * HL:
                                  wi * WSTRIDE + (l + 1) * hs * HL].rearrange(
                            "p (t m) -> p t m", t=hs
                        ),
                    )

            load_w(0, 0, nc.sync)              # wk first
            wk_sb = wkvq_sb[:, 0]
            wv_sb = wkvq_sb[:, 1]
            wq_sb = wkvq_sb[:, 2]
            consts_sb = wp.tile([128, NCON], F32)
            nc.sync.dma_start(consts_sb[:], consts.ap())
            bk_sb = consts_sb[:, 0:1]
            bq_sb = consts_sb[:, 1:2]
            bvb_sb = consts_sb[:, 2:2 + HL]
            maskb_sb = consts_sb[:, 2 + HL:NCON]

            # persistent per-core activations
            qT_sb = bigp.tile([128, T], BF16)      # \hat q^T (2 heads stacked)
            kT_sb = bigp.tile([128, TK], BF16)
            # v natural, TK/128 token-tiles: [v_h0 (64) | 1 | v_h1 (64) | 1]
            v_sb = bigp.tile([128, TK // 128, 130], BF16)
            ctxT_sb = bigp.tile([128, T], BF16)    # normalized ctx^T (2 heads stacked)


            def emit_qkproj(cc, w, w_sb, b_sb, xT, outT, nloads=1):
                """project columns [cc, cc+w) of xT with w_sb -> outT (d-major)."""
                x_t = iop.tile([128, NDT, 512], BF16, tag="xin", bufs=4, name="x_t")
                wl = w // nloads
                for l in range(nloads):
                    nc.gpsimd.dma_start(
                        x_t[:, :, l * wl:(l + 1) * wl],
                        xT.ap()[:, cc + l * wl:cc + (l + 1) * wl].rearrange(
                            "(t p) m -> p t m", p=128
                        ),
                    )
                ps_k = ps.tile([128, 512], F32, tag="mm512", bufs=2, name="ps_k")
                for l in range(nloads):
                    for dt in range(NDT):
                        nc.tensor.matmul(
                            ps_k[:, l * wl:(l + 1) * wl], w_sb[:, dt, :],
                            x_t[:, dt, l * wl:(l + 1) * wl],
                            start=(dt == 0), stop=(dt == NDT - 1),
                        )
                nc.vector.tensor_scalar_add(outT[:, cc:cc + w], ps_k[:, 0:w], b_sb)

            def emit_vproj(cc, w, nloads=1):
                """value projection for compacted tokens [cc, cc+w) -> v_sb."""
                tt0 = cc // 128
                ntt = w // 128
                xv_t = iop.tile([128, NDT, 512], BF16, tag="xvin", bufs=2, name="xv_t")
                wl = w // nloads
                for l in range(nloads):
                    nc.gpsimd.dma_start(
                        xv_t[:, :, l * wl:(l + 1) * wl],
                        xvT.ap()[:, cc + l * wl:cc + (l + 1) * wl].rearrange(
                            "(t p) m -> p t m", p=128
                        ),
                    )
                ps_v = ps.tile([128, 512], F32, tag="mm512", bufs=2, name="ps_v")
                # one accumulation group at a time per PSUM bank: a mid-bank
                # start=True clears has_written for the whole bank
                for tt in range(ntt):
                    col = tt * 128
                    for dt in range(NDT):
                        nc.tensor.matmul(
                            ps_v[:, col:col + 128],
                            xv_t[:, dt, tt * 128:(tt + 1) * 128],
                            wv_sb[:, dt, :],
                            start=(dt == 0), stop=(dt == NDT - 1),
                        )
                for tt in range(ntt):
                    for h in range(2):
                        nc.vector.tensor_add(
                            v_sb[:, tt0 + tt, h * 65: h * 65 + 64],
                            ps_v[:, tt * 128 + h * 64: tt * 128 + h * 64 + 64],
                            bvb_sb[:, h * 64: h * 64 + 64],
                        )

            def emit_attn(b, qb, fillers=None, qhalves=1, between=None):
                fillers = fillers or {}
                qc = b * S + qb * 512
                j = qc // TCHUNK
                qw = 512 // qhalves
                for qh in range(qhalves):
                    # separate tiles per query-slice (tile-granular dependency
                    # tracking would otherwise serialize the slices); odd
                    # slices borrow the projection psum pool, idle by then
                    if qh % 2 == 0:
                        ps_c0 = ps.tile([128, 512], F32, tag="ctx", bufs=2,
                                        name="ps_c0")
                        ps_c1 = ps.tile([128, 512], F32, tag="ctx", bufs=2,
                                        name="ps_c1")
                    else:
                        ps_c0 = ps.tile([128, 512], F32, tag="mm512", bufs=2,
                                        name="ps_c0b")
                        ps_c1 = ps.tile([128, 512], F32, tag="mm512", bufs=2,
                                        name="ps_c1b")
                    qc0 = qc + qh * qw
                    cs = slice(0, qw)
                    es = []

                    def emit_pv(kb):
                        e0, e1, kt = es[kb]
                        nc.tensor.matmul(
                            ps_c0[0:65, cs], v_sb[:, kt, 0:65], e0,
                            start=(kb == 0), stop=(kb == kblocks - 1),
                        )
                        nc.tensor.matmul(
                            ps_c1[0:65, cs], v_sb[:, kt, 65:130], e1,
                            start=(kb == 0), stop=(kb == kblocks - 1),
                        )

                    for kb in range(kblocks):
                        kc = b * SK + kb * 128  # column in kT_sb / v_sb tile
                        kt = kc // 128
                        ps_l = ps.tile([128, 1024], F32, tag="logit", bufs=2,
                                       name="ps_l")
                        e = iop.tile([128, 1024], BF16, tag="exp", bufs=24,
                                     name="e")
                        nc.tensor.matmul(
                            ps_l[:, 0:qw], kT_sb[0:64, kc:kc + 128],
                            qT_sb[0:64, qc0:qc0 + qw],
                        )
                        nc.tensor.matmul(
                            ps_l[:, 512:512 + qw], kT_sb[64:128, kc:kc + 128],
                            qT_sb[64:128, qc0:qc0 + qw],
                        )
                        mcol = b * kblocks + kb
                        eh = e.rearrange("p (h m) -> p h m", h=2)
                        lh = ps_l.rearrange("p (h m) -> p h m", h=2)
                        nc.scalar.activation(
                            eh[:, :, 0:qw], lh[:, :, 0:qw], AFT.Exp,
                            bias=maskb_sb[:, mcol:mcol + 1], scale=0.125,
                        )
                        es.append((e[:, 0:qw], e[:, 512:512 + qw], kt))
                        if kb >= 1:
                            emit_pv(kb - 1)
                        for f in fillers.get((qh, kb), fillers.get(kb, [])
                                             if qh == 0 else []):
                            f()
                    emit_pv(kblocks - 1)
                    # epilogue: normalize by denominators (psum row 64)
                    r0 = iop.tile([1, 512], F32, tag="r0", bufs=4, name="r0")
                    r1 = iop.tile([1, 512], F32, tag="r1", bufs=4, name="r1")
                    nc.vector.reciprocal(r0[:, 0:qw], ps_c0[64:65, cs])
                    nc.vector.reciprocal(r1[:, 0:qw], ps_c1[64:65, cs])
                    rec0 = iop.tile([64, 512], F32, tag="rec0", bufs=4,
                                    name="rec0")
                    rec1 = iop.tile([64, 512], F32, tag="rec1", bufs=4,
                                    name="rec1")
                    nc.gpsimd.partition_broadcast(rec0[:, 0:qw], r0[:, 0:qw])
                    nc.gpsimd.partition_broadcast(rec1[:, 0:qw], r1[:, 0:qw])
                    nc.vector.tensor_mul(
                        ctxT_sb[0:64, qc0:qc0 + qw], ps_c0[0:64, cs],
                        rec0[:, 0:qw],
                    )
                    nc.vector.tensor_mul(
                        ctxT_sb[64:128, qc0:qc0 + qw], ps_c1[0:64, cs],
                        rec1[:, 0:qw],
                    )
                    # eager A2A input staging for the finished token slices
                    if "a2a" in phases:
                        hs = (range(NA2A) if qhalves == 1
                              else range(qh * NA2A // qhalves,
                                         (qh + 1) * NA2A // qhalves))
                        for h in hs:
                            c0 = j * TCHUNK + h * TH
                            nc.sync.dma_start(
                                a2a_in[h].ap()[j][0:64],
                                ctxT_sb[0:64, c0:c0 + TH],
                            )
                            nc.sync.dma_start(
                                a2a_in[h].ap()[j][64:128],
                                ctxT_sb[64:128, c0:c0 + TH],
                            )
                    if between is not None and qh < qhalves - 1:
                        between(qh)

            kv_chunks = [(cc, min(512, TK - cc)) for cc in range(0, TK, 512)]
            nkv = len(kv_chunks)
            q_chunks = [(tb * 512, 512) for tb in range(T // 512)]

            def kf(i):
                cc, w = kv_chunks[i]
                return lambda: emit_qkproj(cc, w, wk_sb, bk_sb, xkT, kT_sb)

            def vf(i):
                cc, w = kv_chunks[i]
                return lambda: emit_vproj(cc, w)

            def qf(i):
                cc, w = q_chunks[i]
                return lambda: emit_qkproj(cc, w, wq_sb, bq_sb, xqT, qT_sb)

            half = nkv // 2    # first b0 kv chunk count
            do = {"k": "kproj" in phases, "v": "vproj" in phases,
                  "q": "qproj" in phases, "a": "attn" in phases}

            for rep in range(reps):
                # minimal prologue: K chunk (split load) + Q chunk (split);
                # wq/wv loads ride the ordered Pool queue between x streams
                if do["k"]:
                    cc, w = kv_chunks[0]
                    emit_qkproj(cc, w, wk_sb, bk_sb, xkT, kT_sb, nloads=2)
                if rep == 0:
                    load_w(2, 2, nc.gpsimd)    # wq
                if do["q"]:
                    cc, w = q_chunks[0]
                    emit_qkproj(cc, w, wq_sb, bq_sb, xqT, qT_sb, nloads=2)
                if rep == 0:
                    load_w(1, 1, nc.gpsimd)    # wv
                if rep == 0:
                    # ones columns for the softmax-denominator rows; needed
                    # only by the first PV, so emitted after the prologue
                    nc.vector.memset(v_sb[:, :, 64], 1.0)
                    nc.vector.memset(v_sb[:, :, 129], 1.0)
                # all other b0 K/V chunks fill the first attention block's
                # activation-paced slack; later chunks spread over the rest
                mid0 = {}
                post = {i: [] for i in range(2 * NQB)}
                if do["v"]:
                    cc0, w0 = kv_chunks[0]
                    mid0[(0, 0)] = [lambda: emit_vproj(cc0, w0, nloads=2)]
                if do["k"] and half > 1:
                    mid0[(0, kblocks // 2 - 1)] = [kf(i) for i in range(1, half)]
                if do["v"] and half > 1:
                    mid0[(0, kblocks // 2)] = [vf(i) for i in range(1, half)]
                if do["q"]:
                    for i in range(1, NQB):
                        post[i - 1].append(qf(i))        # q_i before block i
                    for i in range(NQB, 2 * NQB):
                        post[i - 1].append(qf(i))        # b1 q chunks
                if do["k"]:
                    for n, i in enumerate(range(half, nkv)):
                        post[n].append(kf(i))            # b1 K chunks early
                if do["v"]:
                    for n, i in enumerate(range(half, nkv)):
                        post[2 + n].append(vf(i))        # b1 V chunks
                ctxf = [None] * NA2A

                # NOTE (future work): attempts to batch the last block's exp
                # activations across key-block groups shifted the schedule
                # enough to expose a latent ordering hazard: the a2a round-h
                # copy/collective and the scalar-queue ctxf re-gather loads
                # are not reliably ordered by the framework's DRAM dependency
                # tracking, which surfaced as sparse NaN in ctxf. The shipped
                # schedule wins that race deterministically (the busy scalar
                # queue delays the loads); alternate load queues (sync, pool,
                # late-emission) all cost 3-6us. Any schedule change around
                # the stagings/copies/loads must be re-validated numerically
                # on device (see bisect_nan.py), not just in TimelineSim.
                def emit_a2a(h):
                    if collective:
                        nc.gpsimd.collective_compute(
                            "AllToAll",
                            mybir.AluOpType.bypass,
                            replica_groups=[list(range(N_CORES))],
                            ins=[a2a_in[h].ap().opt()],
                            outs=[a2a_out[h].ap().opt()],
                        )
                    else:  # sim stand-in: same-volume local copy
                        nc.sync.dma_start(a2a_out[h].ap(), a2a_in[h].ap())
                    if "out" in phases:
                        ctxf[h] = bigp.tile([128, N_CORES, TH], BF16,
                                            tag=f"ctxf{h}", name=f"ctxf{h}")
                        for ih in range(2):
                            nc.scalar.dma_start(
                                ctxf[h][:, ih * 4:(ih + 1) * 4, :],
                                a2a_out[h].ap()[ih * 4:(ih + 1) * 4].rearrange(
                                    "i p m -> p i m"
                                ),
                            )

                wo_sb = bob_sb = None

                out_ps = {}

                def emit_out_mms(tt, i):
                    if tt not in out_ps:
                        ps_o = ps.tile([128, 1024], F32, tag="logit",
                                       bufs=2, name="ps_o")
                        out_ps[tt] = (ps_o[:, 0:512], ps_o[:, 512:1024])
                    ps_a, ps_b = out_ps[tt]
                    lhs = ctxf[tt][:, i, 0:128]
                    nc.tensor.matmul(
                        ps_a[:, 0:512], lhs, wo_sb[:, i, 0:512],
                        start=(i == 0), stop=(i == N_CORES - 1),
                    )
                    nc.tensor.matmul(
                        ps_b[:, 0:512], lhs, wo_sb[:, i, 512:1024],
                        start=(i == 0), stop=(i == N_CORES - 1),
                    )

                def emit_out_tt(tt, last_tt=None, epi_only=False):
                    if last_tt is None:
                        last_tt = tt == TCHUNK // 128 - 1
                    if not epi_only:
                        for i in range(N_CORES):
                            emit_out_mms(tt, i)
                    ps_a, ps_b = out_ps[tt]
                    o = iop.tile([128, 1024], F32, tag="osb", bufs=4, name="o")
                    nc.vector.tensor_add(o[:, 0:512], ps_a[:], bob_sb[:, 0:512])
                    nc.sync.dma_start(
                        out.ap()[tt * 128:(tt + 1) * 128, 0:512], o[:, 0:512]
                    )
                    nc.vector.tensor_add(
                        o[:, 512:1024], ps_b[:], bob_sb[:, 512:1024]
                    )
                    nc.sync.dma_start(
                        out.ap()[tt * 128:(tt + 1) * 128, 512:1024],
                        o[:, 512:1024],
                    )

                for blk in range(2 * NQB):
                    if do["a"]:
                        last = blk == 2 * NQB - 1
                        fl = mid0 if blk == 0 else None
                        AH = NA2A   # attention slices for the last block
                        rpa = NA2A // AH   # a2a rounds fired per slice

                        def between(qh):
                            for h in range(qh * rpa, (qh + 1) * rpa):
                                emit_a2a(h)

                        emit_attn(blk // NQB, blk % NQB, fl,
                                  qhalves=AH if last and "a2a" in phases
                                  else 1,
                                  between=between
                                  if last and "a2a" in phases else None)
                    for f in post[blk]:
                        f()
                    if blk == 3 and rep == 0 and "out" in phases:
                        # Wo/bo loads: Pool-queue ordered so the transfers run
                        # here (DMA device idle) and cannot preempt the
                        # prologue x streams
                        wo_sb = wp.tile([128, NDT, D], BF16)
                        bob_sb = wp.tile([128, D], F32)
                        nc.gpsimd.dma_start(
                            wo_sb[:], wo.ap().rearrange("p (t m) -> p t m", t=NDT)
                        )
                        nc.gpsimd.dma_start(bob_sb[:], bob.ap())

                if dump:
                    nc.gpsimd.dma_start(d_qT.ap(), qT_sb[:])
                    nc.gpsimd.dma_start(d_kT.ap(), kT_sb[:])
                    nc.gpsimd.dma_start(d_v.ap(), v_sb[:])
                    nc.gpsimd.dma_start(d_ctxT.ap(), ctxT_sb[:])

                if "a2a" in phases:
                    for h in range((AH - 1) * rpa, NA2A):
                        emit_a2a(h)

                # ---- output projection for my 512-token chunk ----
                if "out" in phases:
                    for tt in range(TCHUNK // 128):
                        emit_out_tt(tt)

    nc.compile()
    return nc


_NC_CACHE = {}


def _get_nc(kblocks, clean_upto=(0, 0)):
    key = (kblocks, tuple(clean_upto))
    if key not in _NC_CACHE:
        _NC_CACHE[key] = build_nc(kblocks=kblocks, clean_upto=clean_upto)
    return _NC_CACHE[key]


# inputs identical on every core -> uploaded once and replicated by XLA
_REPLICATED = {"xqT", "xkT", "xvT", "wo", "bob"}

_RUNNER_CACHE = {}


def _make_runner(nc):
    """Compile a shard_map-wrapped executor for `nc` once; returns
    run(in_maps) -> list of per-core output dicts."""
    import jax
    from jax.sharding import Mesh, NamedSharding, PartitionSpec as P
    from jax.experimental.shard_map import shard_map
    import concourse.bass2jax as b2j

    b2j.install_neuronx_cc_hook()
    in_names, out_names, out_avals = [], [], []
    for alloc in nc.m.functions[0].allocations:
        if not isinstance(alloc, mybir.MemoryLocationSet):
            continue
        name = alloc.memorylocations[0].name
        if alloc.kind == "ExternalInput":
            in_names.append(name)
        elif alloc.kind == "ExternalOutput":
            out_names.append(name)
            out_avals.append(
                jax.core.ShapedArray(
                    tuple(alloc.tensor_shape), mybir.dt.np(alloc.dtype)
                )
            )
    pid_name = nc.partition_id_tensor.name if nc.partition_id_tensor else None
    n_params = len(in_names)
    all_in_names = in_names + out_names

    def _body(*args):
        return tuple(
            b2j._bass_exec_p.bind(
                *args,
                out_avals=tuple(out_avals),
                in_names=tuple(all_in_names),
                out_names=tuple(out_names),
                lowering_input_output_aliases=(),
                sim_require_finite=True,
                sim_require_nnan=True,
                nc=nc,
            )
        )

    devices = jax.devices()[:N_CORES]
    mesh = Mesh(np.asarray(devices), ("core",))

    def spec_for(name):
        return P() if name in _REPLICATED else P("core")

    in_specs = tuple(spec_for(n) for n in in_names) + (P("core"),) * len(out_names)
    out_specs = (P("core"),) * len(out_names)
    fn = jax.jit(
        shard_map(_body, mesh=mesh, in_specs=in_specs, out_specs=out_specs,
                  check_rep=False),
        keep_unused=True,
    )
    sh_core = NamedSharding(mesh, P("core"))
    sh_repl = NamedSharding(mesh, P())
    zero_outs = [
        np.zeros((N_CORES * a.shape[0],) + tuple(a.shape[1:]), a.dtype)
        for a in out_avals
    ]
    upload_cache = {}

    def _put(name, arr, sh):
        import hashlib
        key = hashlib.blake2b(arr.tobytes(), digest_size=16).digest()
        hit = upload_cache.get(name)
        if hit is not None and hit[0] == key:
            return hit[1]
        buf = jax.device_put(arr, sh)
        upload_cache[name] = (key, buf)
        return buf

    def run(in_maps):
        args = []
        for name in in_names:
            if name == pid_name:
                cat = np.arange(N_CORES, dtype=np.uint32).reshape(N_CORES, 1)
                args.append(_put(name, cat, sh_core))
            elif name in _REPLICATED:
                args.append(_put(name, np.asarray(in_maps[0][name]), sh_repl))
            else:
                cat = np.concatenate(
                    [np.asarray(m[name]) for m in in_maps], axis=0
                )
                args.append(_put(name, cat, sh_core))
        for i, z in enumerate(zero_outs):
            args.append(_put(f"__zero{i}", z, sh_core))
        outs = fn(*args)
        jax.block_until_ready(outs)
        res = []
        for c in range(N_CORES):
            d = {}
            for i, name in enumerate(out_names):
                arr = np.asarray(outs[i])
                per = arr.shape[0] // N_CORES
                d[name] = arr[c * per:(c + 1) * per]
            res.append(d)
        return res

    return run


def _get_runner(kblocks, clean_upto=(0, 0)):
    key = (kblocks, tuple(clean_upto))
    if key not in _RUNNER_CACHE:
        _RUNNER_CACHE[key] = _make_runner(_get_nc(kblocks, clean_upto))
    return _RUNNER_CACHE[key]


def prepare_in_maps(kblocks, query, key, value, mask, Wq, bq, Wk, bk, Wv, bv, Wo, bo):
    SK = kblocks * 128
    m = np.asarray(mask, dtype=np.float32).reshape(B, S)
    key2 = np.asarray(key, dtype=np.float32).reshape(T, D)
    val2 = np.asarray(value, dtype=np.float32).reshape(T, D)

    rows = np.zeros(B * SK, np.int64)
    maskb = np.full((128 * kblocks, B), -1e9, np.float32)
    for b in range(B):
        idx = np.flatnonzero(m[b] == 0)
        n = len(idx)
        assert n <= SK, f"unmasked count {n} exceeds capacity {SK}"
        rows[b * SK: b * SK + n] = b * S + idx
        maskb[:n, b] = 0.0
    # maskb[p, b*kblocks+kb] with p = position within block kb
    maskb = np.ascontiguousarray(
        maskb.reshape(kblocks, 128, B).transpose(1, 2, 0).reshape(128, B * kblocks)
    )

    xqT = np.ascontiguousarray(
        np.asarray(query, np.float32).reshape(T, D).T.astype(NPBF16)
    )
    xkT = np.ascontiguousarray(key2[rows].T.astype(NPBF16))
    xvT = np.ascontiguousarray(val2[rows].T.astype(NPBF16))
    Wo_c = np.asarray(Wo, np.float32).astype(NPBF16)
    Wo_c = np.ascontiguousarray(
        Wo_c.reshape(NDT, 128, D).transpose(1, 0, 2).reshape(128, NDT * D)
    )
    bob = np.ascontiguousarray(np.broadcast_to(bo, (128, D)), dtype=np.float32)

    in_maps = []
    for c in range(N_CORES):
        sl = slice(c * HL, (c + 1) * HL)
        # [128, 3*NDT*HL]: per partition [wk(t,m) | wv(t,m) | wq(t,m)]
        wkvq = np.concatenate(
            [
                np.asarray(W[:, sl], np.float32)
                .astype(NPBF16)
                .reshape(NDT, 128, HL)
                .transpose(1, 0, 2)
                .reshape(128, NDT * HL)
                for W in (Wk, Wv, Wq)
            ],
            axis=1,
        )
        consts = np.concatenate(
            [
                np.asarray(bk[sl], np.float32).reshape(128, 1),
                np.asarray(bq[sl], np.float32).reshape(128, 1),
                np.tile(np.asarray(bv[sl], np.float32), (128, 1)),
                maskb,
            ],
            axis=1,
        )
        in_maps.append(
            {
                "xqT": xqT, "xkT": xkT, "xvT": xvT,
                "wkvq": np.ascontiguousarray(wkvq),
                "wo": Wo_c,
                "consts": np.ascontiguousarray(consts),
                "bob": bob,
            }
        )
    return in_maps


def _pick_kblocks(mask):
    m = np.asarray(mask).reshape(B, S)
    counts = (m == 0).sum(axis=1)
    maxn = int(counts.max())
    kblocks = min(S // 128, max(1, math.ceil(maxn / 128)))
    # clean_upto is currently unused by the emitted program (the grouped
    # zero-bias exp variant was reverted after failing on hardware); keep the
    # plumbing inert with a constant so the compile cache stays stable.
    clean_upto = (0, 0)
    return kblocks, clean_upto


def kernel(**inputs) -> np.ndarray:
    kblocks, clean_upto = _pick_kblocks(inputs["mask"])
    in_maps = prepare_in_maps(kblocks, **inputs)
    try:
        run = _get_runner(kblocks, clean_upto)
        results = run(in_maps)
    except Exception:
        # robust fallback: the stock SPMD runner
        res = bass_utils.run_bass_kernel_spmd(
            _get_nc(kblocks, clean_upto), in_maps, core_ids=list(range(N_CORES))
        )
        results = res.results
    out = np.concatenate([results[c]["out"] for c in range(N_CORES)], axis=0)
    return out.reshape(B, S, D)
